# revision 1
# baseline (speedup 1.0000x reference)
"""Trainium2 Bass kernel for nn_ConditionalSplineFlow (8-core data parallel).

Layout strategy:
  - MLP runs in "transposed world": activations [feature, rows] so weight
    matrices act as lhsT directly; final GEMM flips orientation using h2^T as
    lhsT, giving params [rows(part), 1472(free)] with W3 columns reordered to
    [uw(d-major,k-inner) | uh | ud(j-major)].
  - Spline is elementwise in (row, dim) with bins innermost; bin search +
    gathers via monotone-mask predicated-copy walks.
  - LU layer folded to  x' = y @ (L@U)^T + b  via PE transpose + matmul.
  - Per-layer logdet of LU and the gaussian constant are folded on host.
"""
import os
import numpy as np
from contextlib import ExitStack

import concourse.bass as bass
import concourse.bacc as bacc
import concourse.tile as tile
import concourse.mybir as mybir
from concourse import bass_utils
from concourse.masks import make_identity

# Pin all activations to the one table set that covers Exp/Ln/Relu/Copy/Abs —
# the default per-function chooser ping-pongs between sets (~2.6us per swap,
# once per row-tile). Masking the other sets (order preserved, so positional
# set ids stay valid) forces a single resident table.
_PINNED_ACT_SET = "natural_log_exp_and_others"
_orig_gat = bacc.get_activation_tables


def _gat_pinned(arch):
    tabs = _orig_gat(arch)
    return {name: (fns if name == _PINNED_ACT_SET else set())
            for name, fns in tabs.items()}


bacc.get_activation_tables = _gat_pinned

F32 = mybir.dt.float32
F16 = mybir.dt.float16
AF = mybir.ActivationFunctionType
OP = mybir.AluOpType

D = 64
NB = 8
L = 5
HID = 256
ODIM = 1472
BND = 5.0
MIN_W = 1e-3
MIN_D = 1e-3
SCL = 2.0 * BND * (1.0 - MIN_W * NB)      # 9.92
SPBOUND = 1.0 - MIN_D                      # softplus value at padded boundary
N_CORES = 8
BATCH = 32768

MM_DT = F16   # matmul dtype for MLP


def build_program(rpc, has_b3=False, bin_f16=False):
    """Build the single-core program for `rpc` rows. Returns (nc, names)."""
    nc = bacc.Bacc(
        "TRN2", target_bir_lowering=False, debug=False,
        enable_asserts=False, num_devices=N_CORES,
    )
    NT = rpc // 128               # row tiles
    CHW = min(rpc, 1024)          # mlp chunk width (rows)
    NCH = rpc // CHW              # chunks
    RT_PER_CH = CHW // 128
    BT = F16 if bin_f16 else F32  # bin-space dtype

    # ---------------- DRAM I/O ----------------
    xin_d = nc.dram_tensor("xin", [rpc, D], F32, kind="ExternalInput").ap()
    ctxT_d = nc.dram_tensor("ctxT", [128, rpc], MM_DT, kind="ExternalInput").ap()
    w1_d = nc.dram_tensor("w1", [L, 128, HID], MM_DT, kind="ExternalInput").ap()
    w2_d = nc.dram_tensor("w2", [L, 2, 128, HID], MM_DT, kind="ExternalInput").ap()
    w3_d = nc.dram_tensor("w3", [L, 2, 128, ODIM], MM_DT, kind="ExternalInput").ap()
    b1_d = nc.dram_tensor("bias1", [L, 2, 128], F32, kind="ExternalInput").ap()
    b2_d = nc.dram_tensor("bias2", [L, 2, 128], F32, kind="ExternalInput").ap()
    at_d = nc.dram_tensor("at", [L, D, D], MM_DT, kind="ExternalInput").ap()
    lub_d = nc.dram_tensor("lub", [L, D], F32, kind="ExternalInput").ap()
    ce_d = nc.dram_tensor("cedge", [7, D], BT, kind="ExternalInput").ap()
    if has_b3:
        b3_d = nc.dram_tensor("bias3", [1, ODIM], F32, kind="ExternalInput").ap()
    out_d = nc.dram_tensor("out", [rpc], F32, kind="ExternalOutput").ap()

    with tile.TileContext(nc) as tc, ExitStack() as ctx:
        # ---------------- pools ----------------
        singles = ctx.enter_context(tc.tile_pool(name="singles", bufs=1))
        wpool = ctx.enter_context(tc.tile_pool(name="wpool", bufs=2))
        hpool = ctx.enter_context(tc.tile_pool(name="hpool", bufs=2))
        binp = ctx.enter_context(tc.tile_pool(name="binp", bufs=2))
        frm = ctx.enter_context(tc.tile_pool(name="frm", bufs=1))
        gcp = ctx.enter_context(tc.tile_pool(name="gcp", bufs=2))
        ps_par = ctx.enter_context(tc.tile_pool(name="ps_par", bufs=1, space="PSUM"))
        ps_h = ctx.enter_context(tc.tile_pool(name="ps_h", bufs=1, space="PSUM"))
        ps_lu = ctx.enter_context(tc.tile_pool(name="ps_lu", bufs=1, space="PSUM"))

        # ---------------- resident tiles ----------------
        ident = singles.tile([128, 128], F32)
        make_identity(nc, ident)
        ctxT = singles.tile([128, rpc], MM_DT)
        nc.sync.dma_start(out=ctxT, in_=ctxT_d)
        # x ping-pong, resident across a layer
        xa = singles.tile([128, NT, D], F32, tag="xa")
        xb = singles.tile([128, NT, D], F32, tag="xb")
        nc.sync.dma_start(out=xa, in_=xin_d.rearrange("(t p) d -> p t d", p=128))
        # logdet accumulator ping-pong [128, NT]
        lda = singles.tile([128, NT], F32, tag="lda")
        ldb = singles.tile([128, NT], F32, tag="ldb")
        # edge constants c_j broadcast [128, 7, 64]
        cedge = singles.tile([128, 7, D], BT)
        nc.sync.dma_start(
            out=cedge,
            in_=bass.AP(tensor=ce_d.tensor, offset=0, ap=[[0, 128], [D, 7], [1, D]]),
        )
        if has_b3:
            ones1 = singles.tile([1, 128], F32)
            nc.vector.memset(ones1, 1.0)
            b3sb = singles.tile([1, ODIM], F32)
            nc.sync.dma_start(out=b3sb, in_=b3_d)
        # persistent edge/cumheight/softplus tiles; const slots written once
        NVB = 3
        Vbufs = [singles.tile([128, 9, 3, D], BT, name=f"Vb{i}") for i in range(NVB)]
        for i in range(NVB):
            nc.gpsimd.memset(Vbufs[i][:, 0, 0:2, :], -BND)
            nc.gpsimd.memset(Vbufs[i][:, 8, 0:2, :], BND)
            nc.gpsimd.memset(Vbufs[i][:, 0, 2, :], SPBOUND)
            nc.gpsimd.memset(Vbufs[i][:, 8, 2, :], SPBOUND)

        x_cur, x_nxt = xa, xb
        ld_cur, ld_nxt = lda, ldb

        for l in range(L):
            # ---------------- layer weights ----------------
            w1t = wpool.tile([128, HID], MM_DT, tag="w1")
            w2t = wpool.tile([128, 2, HID], MM_DT, tag="w2")
            w3t = wpool.tile([128, 2, ODIM], MM_DT, tag="w3")
            b1t = wpool.tile([128, 2], F32, tag="b1")
            b2t = wpool.tile([128, 2], F32, tag="b2")
            att = wpool.tile([64, D], MM_DT, tag="at")
            lubt = wpool.tile([128, D], F32, tag="lub")
            nc.sync.dma_start(out=w1t, in_=w1_d[l])
            nc.sync.dma_start(out=w2t, in_=w2_d[l].rearrange("k p h -> p k h"))
            nc.sync.dma_start(out=w3t, in_=w3_d[l].rearrange("k p h -> p k h"))
            nc.sync.dma_start(out=b1t, in_=b1_d[l].rearrange("t p -> p t"))
            nc.sync.dma_start(out=b2t, in_=b2_d[l].rearrange("t p -> p t"))
            nc.sync.dma_start(out=att, in_=at_d[l])
            nc.sync.dma_start(
                out=lubt,
                in_=bass.AP(tensor=lub_d.tensor, offset=l * D,
                            ap=[[0, 128], [1, D]]),
            )

            for chi in range(NCH):
                # ---------------- MLP chunk (transposed world) ----------------
                c0 = chi * CHW
                NHALF = max(1, CHW // 512)
                h1t = hpool.tile([128, 2, CHW], MM_DT, tag="h1")
                for m in range(2):
                    ps1 = ps_h.tile([128, CHW], F32, tag="psh")
                    for hf in range(NHALF):
                        h0 = hf * 512
                        hw_ = min(512, CHW - h0)
                        nc.tensor.matmul(
                            ps1[:, h0:h0 + hw_],
                            lhsT=w1t[:, m * 128:(m + 1) * 128],
                            rhs=ctxT[:, c0 + h0:c0 + h0 + hw_])
                    nc.scalar.activation(h1t[:, m, :], ps1, AF.Relu,
                                         bias=b1t[:, m:m + 1])
                h2t = hpool.tile([128, 2, CHW], MM_DT, tag="h2")
                for m in range(2):
                    ps2 = ps_h.tile([128, CHW], F32, tag="psh")
                    for hf in range(NHALF):
                        h0 = hf * 512
                        hw_ = min(512, CHW - h0)
                        for kk in range(2):
                            nc.tensor.matmul(
                                ps2[:, h0:h0 + hw_],
                                lhsT=w2t[:, kk, m * 128:(m + 1) * 128],
                                rhs=h1t[:, kk, h0:h0 + hw_],
                                start=(kk == 0), stop=(kk == 1))
                    nc.scalar.activation(h2t[:, m, :], ps2, AF.Relu,
                                         bias=b2t[:, m:m + 1])

                FRT = RT_PER_CH
                for sub in range(RT_PER_CH // FRT):
                    # ---- per-chunk gather targets + clipped x ----
                    RT4 = FRT
                    rt0 = chi * RT_PER_CH + sub * FRT
                    Gc = gcp.tile([128, RT4, 6, D], BT, tag="Gc", name="Gc")
                    xcc = gcp.tile([128, RT4, D], F32, tag="xcc", name="xcc")
                    nc.vector.tensor_scalar(xcc, x_cur[:, rt0:rt0 + RT4, :],
                                            -BND, BND, OP.max, OP.min)

                    for rti in range(FRT):
                        rt = rt0 + rti
                        r0 = (sub * FRT + rti) * 128
                        # ---------------- GEMM3: params [128 rows, 1472] ----------
                        psp = ps_par.tile([128, ODIM], F32, tag="pspar")
                        nslices = [(0, 512), (512, 512), (1024, 448)]
                        for (ns, nw) in nslices:
                            for kk in range(2):
                                nc.tensor.matmul(
                                    psp[:, ns:ns + nw],
                                    lhsT=h2t[:, kk, r0:r0 + 128],
                                    rhs=w3t[:, kk, ns:ns + nw],
                                    start=(kk == 0),
                                    stop=(kk == 1) if not has_b3 else False,
                                )
                            if has_b3:
                                nc.tensor.matmul(
                                    psp[:, ns:ns + nw], lhsT=ones1,
                                    rhs=b3sb[:, ns:ns + nw],
                                    start=False, stop=True)

                        # ---------------- spline bin space ----------------
                        # exps k-major [128, q, k, d]; ACT writes via transposed view
                        exps = binp.tile([128, 2, NB, D], BT, tag="exps")
                        nc.scalar.activation(exps[:, 0].transpose([0, 2, 1]),
                                             psp[:, 0:512], AF.Exp)
                        nc.scalar.activation(exps[:, 1].transpose([0, 2, 1]),
                                             psp[:, 512:1024], AF.Exp)
                        # softplus(ud) = Ln(exp(ud) + 1) into V sp-slots
                        V = Vbufs[rt % NVB]
                        expu = binp.tile([128, 7, D], BT, tag="expu")
                        nc.scalar.activation(expu, psp[:, 1024:1472], AF.Exp)
                        nc.scalar.activation(V[:, 1:8, 2, :], expu, AF.Ln, bias=1.0)

                        # psum chains (w,h) on gpsimd: pw[:,q,j,:]; slot 7 = S
                        pw = binp.tile([128, 2, NB, D], BT, tag="pw")
                        nc.gpsimd.tensor_copy(pw[:, :, 0, :], exps[:, :, 0, :])
                        for j in range(1, 8):
                            nc.gpsimd.tensor_add(pw[:, :, j, :], pw[:, :, j - 1, :],
                                                 exps[:, :, j, :])

                        # inv' = SCL / S  (recip on fp32)
                        Spair = frm.tile([128, 2, D], F32, tag="Spair", name="Spair")
                        nc.vector.tensor_copy(Spair, pw[:, :, 7, :])
                        invp = frm.tile([128, 2, D], F32, tag="invp", name="invp")
                        nc.vector.reciprocal_approx_fast(out=invp, in_=Spair)
                        invs = frm.tile([128, 2, D], BT, tag="invs", name="invs")
                        nc.vector.tensor_scalar_mul(invs, invp, SCL)

                        # edges V[:, j, 0:2, :]  j=1..7 interior
                        pe = binp.tile([128, 7, 2, D], BT, tag="pe")
                        nc.vector.tensor_mul(
                            pe,
                            pw[:, :, 0:7, :].transpose([0, 2, 1, 3]),
                            invs.unsqueeze(1).to_broadcast([128, 7, 2, D]))
                        nc.vector.tensor_add(
                            V[:, 1:8, 0:2, :], pe,
                            cedge.unsqueeze(2).to_broadcast([128, 7, 2, D]))

                        # masks (u8, broadcast at use sites)
                        su = binp.tile([128, 7, D], mybir.dt.uint8, tag="su")
                        nc.vector.tensor_tensor(
                            su, xcc[:, rti:rti + 1, :].to_broadcast([128, 7, D]),
                            V[:, 1:8, 0, :], OP.is_ge)

                        # unified walk [le,lch,lsp, re,rch,rsp] into Gc[:, rti]
                        nc.vector.tensor_copy(Gc[:, rti], V[:, 0:2, :, :])
                        for j in range(1, 8):
                            nc.vector.copy_predicated(
                                Gc[:, rti],
                                su[:, j - 1:j, :].to_broadcast([128, 6, D]),
                                V[:, j:j + 2, :, :])

                    # ---------------- formula (chunked, fp32) ----------------
                    FSH = [128, RT4, D]
                    le, lch = Gc[:, :, 0, :], Gc[:, :, 1, :]
                    re_, rch = Gc[:, :, 3, :], Gc[:, :, 4, :]
                    Gdp = frm.tile([128, RT4, 2, D], F32, tag="Gdp", name="Gdp")
                    nc.gpsimd.tensor_scalar_add(
                        Gdp[:, :, 0, :], Gc[:, :, 2, :], MIN_D)
                    nc.gpsimd.tensor_scalar_add(
                        Gdp[:, :, 1, :], Gc[:, :, 5, :], MIN_D)
                    ind, ind1 = Gdp[:, :, 0, :], Gdp[:, :, 1, :]

                    def ft(tag):
                        return frm.tile(FSH, F32, tag=tag, name=tag)

                    xt = x_cur[:, rt0:rt0 + RT4, :]
                    in_w = ft("in_w"); nc.vector.tensor_sub(in_w, re_, le)
                    rw = ft("rw"); nc.vector.reciprocal_approx_fast(out=rw, in_=in_w)
                    tnum = ft("tnum"); nc.vector.tensor_sub(tnum, xcc, le)
                    th = ft("th"); nc.vector.tensor_mul(th, tnum, rw)
                    in_h = ft("in_h"); nc.vector.tensor_sub(in_h, rch, lch)
                    idel = ft("idel"); nc.vector.tensor_mul(idel, in_h, rw)
                    omt = ft("omt")
                    nc.vector.tensor_scalar(omt, th, -1.0, 1.0, OP.mult, OP.add)
                    tomt = ft("tomt"); nc.vector.tensor_mul(tomt, th, omt)
                    th2 = ft("th2"); nc.vector.tensor_mul(th2, th, th)
                    t1 = ft("t1"); nc.vector.tensor_mul(t1, idel, th2)
                    t2 = ft("t2"); nc.vector.tensor_mul(t2, ind, tomt)
                    nsum = ft("nsum"); nc.vector.tensor_add(nsum, t1, t2)
                    numer = ft("numer"); nc.vector.tensor_mul(numer, in_h, nsum)
                    dd = ft("dd"); nc.vector.tensor_add(dd, ind, ind1)
                    dd2 = ft("dd2")
                    nc.vector.scalar_tensor_tensor(dd2, idel, -2.0, dd,
                                                   OP.mult, OP.add)
                    dt = ft("dt"); nc.vector.tensor_mul(dt, dd2, tomt)
                    denom = ft("denom"); nc.vector.tensor_add(denom, idel, dt)
                    rden = ft("rden")
                    nc.vector.reciprocal_approx_fast(out=rden, in_=denom)
                    yq = ft("yq"); nc.vector.tensor_mul(yq, numer, rden)
                    y = ft("y"); nc.vector.tensor_add(y, lch, yq)
                    omt2 = ft("omt2"); nc.vector.tensor_mul(omt2, omt, omt)
                    u1 = ft("u1"); nc.vector.tensor_mul(u1, th2, ind1)
                    idt = ft("idt"); nc.vector.tensor_mul(idt, idel, tomt)
                    u2 = ft("u2")
                    nc.vector.scalar_tensor_tensor(u2, idt, 2.0, u1,
                                                   OP.mult, OP.add)
                    u3 = ft("u3"); nc.vector.tensor_mul(u3, ind, omt2)
                    uu = ft("uu"); nc.vector.tensor_add(uu, u2, u3)
                    idel2 = ft("idel2"); nc.vector.tensor_mul(idel2, idel, idel)
                    dnum = ft("dnum"); nc.vector.tensor_mul(dnum, uu, idel2)
                    lnd = ft("lnd"); nc.scalar.activation(lnd, dnum, AF.Ln)
                    lnden = ft("lnden"); nc.scalar.activation(lnden, denom, AF.Ln)
                    ldt = ft("ldt")
                    nc.vector.scalar_tensor_tensor(ldt, lnden, -2.0, lnd,
                                                   OP.mult, OP.add)
                    # inside mask + select
                    absx = ft("absx")
                    nc.scalar.activation(absx, xt, AF.Abs)
                    ins = ft("ins")
                    nc.vector.tensor_scalar(ins, absx, BND, None, OP.is_le)
                    insu = frm.tile([128, RT4, D], mybir.dt.uint8, tag="insu",
                                    name="insu")
                    nc.vector.tensor_copy(insu, ins)
                    yfin = ft("yfin")
                    nc.vector.tensor_copy(yfin, xt)
                    nc.vector.copy_predicated(yfin, insu, y)
                    # masked logdet reduce, chained across layers
                    ldm = ft("ldm")
                    nc.vector.tensor_mul(ldm, ldt, ins)
                    ldr = frm.tile([128, RT4], F32, tag="ldr", name="ldr")
                    nc.vector.tensor_reduce(ldr, ldm, mybir.AxisListType.X, OP.add)
                    if l == 0:
                        nc.vector.tensor_copy(ld_nxt[:, rt0:rt0 + RT4], ldr)
                    else:
                        nc.vector.tensor_add(ld_nxt[:, rt0:rt0 + RT4], ldr,
                                             ld_cur[:, rt0:rt0 + RT4])

                    # ---------------- LU per row-tile: x' = y @ A^T + b ---------
                    for rti in range(FRT):
                        rt = rt0 + rti
                        pst = ps_lu.tile([64, 128], F32, tag="pst")
                        nc.tensor.transpose(pst, yfin[:, rti, :], ident)
                        yT = frm.tile([64, 128], MM_DT, tag="yT", name="yT")
                        nc.scalar.copy(yT, pst)
                        psx = ps_lu.tile([128, D], F32, tag="psx")
                        nc.tensor.matmul(psx, lhsT=yT, rhs=att)
                        nc.vector.tensor_add(x_nxt[:, rt, :], psx, lubt)

            x_cur, x_nxt = x_nxt, x_cur
            ld_cur, ld_nxt = ld_nxt, ld_cur

        # ---------------- final: out = -0.5*sum(x^2) + ld + const ----------
        xsq = singles.tile([128, NT, D], F32)
        nc.vector.tensor_mul(xsq, x_cur, x_cur)
        sq = singles.tile([128, NT], F32)
        nc.vector.tensor_reduce(sq, xsq, mybir.AxisListType.X, OP.add)
        ov = singles.tile([128, NT], F32)
        nc.vector.scalar_tensor_tensor(ov, sq, -0.5, ld_cur, OP.mult, OP.add)
        # const added on host (exact); DMA out
        nc.sync.dma_start(out=out_d.rearrange("(t p) -> p t", p=128), in_=ov)

    nc.compile()
    return nc


# ------------------------- host side -------------------------

def _host_prep(inputs):
    x = np.ascontiguousarray(inputs["inputs"].astype(np.float32))
    ctx = inputs["context"].astype(np.float32)
    W1 = inputs["W1"].astype(np.float32)
    W2 = inputs["W2"].astype(np.float32)
    W3 = inputs["W3"].astype(np.float32)
    b1 = inputs["b1"].astype(np.float32)
    b2 = inputs["b2"].astype(np.float32)
    b3 = inputs["b3"].astype(np.float32)

    mmnp = np.float16 if MM_DT == F16 else np.float32

    cols = np.arange(D * 23).reshape(D, 23)
    perm = np.concatenate([
        cols[:, 0:8].reshape(-1),
        cols[:, 8:16].reshape(-1),
        cols[:, 16:23].T.reshape(-1),
    ])
    W3r = W3[:, :, perm]
    b3r = b3[:, perm]
    has_b3 = bool(np.any(b3r != 0.0))

    ctxT = np.ascontiguousarray(ctx.T.astype(mmnp))                 # [128, B]
    w1 = np.ascontiguousarray(W1.astype(mmnp))                      # [L,128,256]
    w2 = np.ascontiguousarray(
        W2.reshape(L, 2, 128, HID).astype(mmnp))                    # [L,2,128,256]
    w3 = np.ascontiguousarray(
        W3r.reshape(L, 2, 128, ODIM).astype(mmnp))                  # [L,2,128,1472]
    bias1 = np.ascontiguousarray(b1.reshape(L, 2, 128))
    bias2 = np.ascontiguousarray(b2.reshape(L, 2, 128))

    li = np.tril_indices(D, -1)
    ui = np.triu_indices(D, 1)
    at = np.zeros((L, D, D), np.float32)
    lld = 0.0
    for l in range(L):
        Lm = np.eye(D, dtype=np.float64)
        Lm[li] = inputs["lower_entries"][l].astype(np.float64)
        diag = np.log1p(np.exp(inputs["upper_diag"][l].astype(np.float64))) + 1e-3
        Um = np.zeros((D, D), np.float64)
        Um[ui] = inputs["upper_entries"][l].astype(np.float64)
        Um += np.diag(diag)
        at[l] = (Lm @ Um).T.astype(np.float32)  # cast to mm dtype below
        lld += float(np.sum(np.log(diag)))
    lub = np.ascontiguousarray(inputs["lu_bias"].astype(np.float32))

    cj = ((2.0 * BND * MIN_W) * np.arange(1, 8).astype(np.float32) - BND)
    cedge = np.ascontiguousarray(
        np.broadcast_to(cj[:, None], (7, D)).astype(np.float32))

    const_total = -0.5 * D * float(np.log(2.0 * np.pi)) + lld

    shared = dict(w1=w1, w2=w2, w3=w3, bias1=bias1, bias2=bias2,
                  at=at.astype(mmnp), lub=lub, cedge=cedge)
    if has_b3:
        shared["bias3"] = np.ascontiguousarray(b3r.reshape(1, ODIM))
    return x, ctxT, shared, has_b3, const_total


_CACHE = {}


def kernel(**inputs):
    rpc = inputs["inputs"].shape[0] // N_CORES
    x, ctxT, shared, has_b3, const_total = _host_prep(inputs)

    bin_f16 = os.environ.get("SPLINE_BIN_F16", "1") == "1"
    key = (rpc, has_b3, bin_f16)
    if key not in _CACHE:
        _CACHE[key] = build_program(rpc, has_b3=has_b3, bin_f16=bin_f16)
    nc = _CACHE[key]

    if bin_f16:
        shared = dict(shared)
        shared["cedge"] = shared["cedge"].astype(np.float16)
    in_maps = []
    for c in range(N_CORES):
        m = dict(shared)
        m["xin"] = np.ascontiguousarray(x[c * rpc:(c + 1) * rpc])
        m["ctxT"] = np.ascontiguousarray(ctxT[:, c * rpc:(c + 1) * rpc])
        in_maps.append(m)

    res = bass_utils.run_bass_kernel_spmd(nc, in_maps, core_ids=list(range(N_CORES)))
    out = np.concatenate([r["out"] for r in res.results])
    return (out + np.float32(const_total)).astype(np.float32)

